# revision 4
# baseline (speedup 1.0000x reference)
"""Conv2d 3x3 (im2col GEMM) on 8 TRN2 NeuronCores.

Problem: x[16,64,112,112] (*) w[576,64] + b[64] -> out[16,64,112,112]
(3x3, stride 1, pad 1, NCHW, im2col patch order (c, kh, kw)).

Strategy
--------
Data-parallel over batch: 2 images per core, 8 cores, no collectives.

Per image, an implicit-GEMM formulation that needs only 3 full-width
fp32r matmuls per 448 outputs (vs 9 for naive per-tap GEMM):

  * x is staged in SBUF as z[128, F]: partitions 0:64 hold the image
    flattened row-major with a 113-element zero pad at each end
    ("zt"), partitions 64:128 hold the same data shifted left by one
    element ("zb", built by an SBUF->SBUF DMA).
  * For each kh in {0,1,2} one matmul with a block lhsT
        [[w(kh,1), w(kh,0)],
         [w(kh,2),    0   ]]
    accumulates into psum[128, 449]:
      psum[0:64,  j] += taps (kw=1 via zt) + (kw=2 via zb)  of out[s+j]
      psum[64:128,j] += tap  (kw=0 via zt)                  of out[s+j+1]
  * ACT adds bias to psum[64:128], DVE folds the two halves ->
    complete conv outputs.
  * Row-major flattening wraps at image-row boundaries, so the kw=0/2
    taps of the first/last column of each row pick up a neighbor-row
    value; two tiny strided matmuls per 28-row group recompute exactly
    those terms (reading the same SBUF words) and DVE subtracts them.

Inputs are pre-rounded on the host to the fp32r grid (11-bit mantissa)
so every device-side producer of matmul data is a pure bit-copy, which
walrus' fp32r verifier accepts; psum accumulation stays full fp32.
"""

import numpy as np

import concourse.bacc as bacc
import concourse.mybir as mybir
import concourse.tile as tile
from concourse import bass_utils

# problem geometry (hardcoded per contract)
B, CIN, H, W = 16, 64, 112, 112
COUT = 64
NCORES = 8
IMGS = B // NCORES  # images per core

HW = H * W                     # 12544
ZOFF = W + 1                   # lead zero pad: 1 + one full pad row
F = ZOFF + HW + ZOFF           # z free size per image (12770)
ROWS_PER_CHUNK = 4
CHUNK = ROWS_PER_CHUNK * W     # 448 outputs per psum chunk
NCHUNK = H // ROWS_PER_CHUNK   # 28
GROUP_CHUNKS = 7               # chunks per output store group
GROUP_ROWS = GROUP_CHUNKS * ROWS_PER_CHUNK   # 28 rows
GROUP = GROUP_CHUNKS * CHUNK   # 3136 outputs
NGROUP = NCHUNK // GROUP_CHUNKS  # 4
# z alloc pad so the (sliced-then-strided) correction rhs views stay in
# bounds; the strided APs themselves never read past F.
F_ALLOC = F + 111

f32 = mybir.dt.float32
f32r = mybir.dt.float32r
u32 = mybir.dt.uint32

_cache = {}


def _round_f32r(a: np.ndarray) -> np.ndarray:
    """Round fp32 to the fp32r grid (11 mantissa bits, RNE)."""
    u = np.ascontiguousarray(a, dtype=np.float32).view(np.uint32).copy()
    lsb = (u >> 12) & 1
    u += 0x7FF + lsb
    u &= np.uint32(0xFFFFF000)
    return u.view(np.float32)


def _build():
    nc = bacc.Bacc("TRN2", target_bir_lowering=False, debug=False,
                   num_devices=NCORES)

    x_d = nc.dram_tensor("x", (IMGS, CIN, H, W), f32r, kind="ExternalInput")
    w_d = nc.dram_tensor("weight", (CIN * 9, COUT), f32r, kind="ExternalInput")
    b_d = nc.dram_tensor("bias", (COUT,), f32, kind="ExternalInput")
    o_d = nc.dram_tensor("out", (IMGS, COUT, H, W), f32, kind="ExternalOutput")

    xv = x_d.ap().rearrange("b c h w -> b c (h w)")
    ov = o_d.ap().rearrange("b c h w -> b c (h w)")
    wv = w_d.ap().rearrange("(c r) m -> c r m", r=9)

    with tile.TileContext(nc) as tc:
        with (
            tc.tile_pool(name="wpool", bufs=1) as wpool,
            tc.tile_pool(name="zpool", bufs=2) as zpool,
            tc.tile_pool(name="opool", bufs=3) as opool,
            tc.tile_pool(name="tpool", bufs=4) as tpool,
            tc.tile_pool(name="ppool", bufs=4, space="PSUM") as ppool,
            tc.tile_pool(name="cpool", bufs=2, space="PSUM") as cpool,
        ):
            # --- weights / bias staging (once) ---
            bias = wpool.tile([COUT, 1], f32)
            nc.sync.dma_start(
                bias[:, :], b_d.ap().rearrange("(c one) -> c one", one=1))

            lhs = []
            w2 = []
            for kh in range(3):
                lt = wpool.tile([128, 128], f32r, name=f"lhsT{kh}",
                                tag=f"lhsT{kh}")
                nc.vector.memset(lt[:, :].bitcast(u32), 0)
                nc.sync.dma_start(lt[0:64, 0:64], wv[:, kh * 3 + 1, :])
                nc.sync.dma_start(lt[0:64, 64:128], wv[:, kh * 3 + 0, :])
                nc.sync.dma_start(lt[64:128, 0:64], wv[:, kh * 3 + 2, :])
                lhs.append(lt)
                w2t = wpool.tile([64, 64], f32r, name=f"w2_{kh}",
                                 tag=f"w2_{kh}")
                nc.sync.dma_start(w2t[:, :], wv[:, kh * 3 + 2, :])
                w2.append(w2t)

            for img in range(IMGS):
                z = zpool.tile([128, F_ALLOC], f32r, name="z", tag="z")
                # zero pads (whole 128 partitions; zb copy re-covers most)
                nc.vector.memset(z[:, 0:ZOFF].bitcast(u32), 0)
                nc.vector.memset(z[:, ZOFF + HW: F].bitcast(u32), 0)
                # zt: the image, flat row-major
                nc.sync.dma_start(z[0:64, ZOFF: ZOFF + HW], xv[img])
                # zb: zt shifted left by one element
                nc.sync.dma_start(z[64:128, 0: F - 1], z[0:64, 1:F])

                for g in range(NGROUP):
                    og = opool.tile([COUT, GROUP], f32, name="og", tag="og")
                    for cc in range(GROUP_CHUNKS):
                        c = g * GROUP_CHUNKS + cc
                        y0 = c * ROWS_PER_CHUNK
                        ps = ppool.tile([128, CHUNK + 2], f32, name="ps",
                                        tag="ps")
                        for kh in range(3):
                            a = (y0 + kh) * W
                            nc.tensor.matmul(
                                ps[:, :],
                                lhs[kh][:, :],
                                z[:, a: a + CHUNK + 2],
                                start=(kh == 0),
                                stop=(kh == 2),
                            )
                        tb = tpool.tile([COUT, CHUNK], f32, name="tb",
                                        tag="tb")
                        nc.scalar.add(tb[:, :], ps[64:128, 0:CHUNK],
                                      bias[:, :])
                        nc.vector.tensor_add(
                            og[:, cc * CHUNK: (cc + 1) * CHUNK],
                            ps[0:64, 1: CHUNK + 1],
                            tb[:, :],
                        )

                    # --- edge corrections for rows y = 28g .. 28g+27 ---
                    pc1 = cpool.tile([64, GROUP_ROWS], f32, name="pc1",
                                     tag="pc1")
                    pc2 = cpool.tile([64, GROUP_ROWS], f32, name="pc2",
                                     tag="pc2")
                    for kh in range(3):
                        a = (GROUP_ROWS * g + kh) * W
                        rhs = z[0:64, a: a + GROUP_ROWS * W].rearrange(
                            "p (r w) -> p r w", w=W)[:, :, 0]
                        nc.tensor.matmul(
                            pc1[:, :], lhs[kh][0:64, 64:128], rhs,
                            start=(kh == 0), stop=(kh == 2))
                    for kh in range(3):
                        a = ZOFF + (GROUP_ROWS * g + kh) * W
                        rhs = z[0:64, a: a + GROUP_ROWS * W].rearrange(
                            "p (r w) -> p r w", w=W)[:, :, 0]
                        nc.tensor.matmul(
                            pc2[:, :], w2[kh][:, :], rhs,
                            start=(kh == 0), stop=(kh == 2))
                    ogv = og[:, :].rearrange("p (r w) -> p r w", w=W)
                    col0 = ogv[:, :, 0]
                    col_last = ogv[:, :, W - 1]
                    nc.vector.tensor_sub(col0, col0, pc1[:, :])
                    nc.vector.tensor_sub(col_last, col_last, pc2[:, :])

                    nc.sync.dma_start(
                        ov[img, :, g * GROUP: (g + 1) * GROUP], og[:, :])

    nc.compile()
    return nc


def kernel(x: np.ndarray, weight: np.ndarray, bias: np.ndarray,
           **_ignored) -> np.ndarray:
    if "nc" not in _cache:
        _cache["nc"] = _build()
    nc = _cache["nc"]

    x_r = _round_f32r(x).reshape(B, CIN, H, W)
    w_r = _round_f32r(weight).reshape(CIN * 9, COUT)
    b_np = np.ascontiguousarray(bias, dtype=np.float32)

    in_maps = [
        {
            "x": np.ascontiguousarray(x_r[i * IMGS: (i + 1) * IMGS]),
            "weight": w_r,
            "bias": b_np,
        }
        for i in range(NCORES)
    ]
    res = bass_utils.run_bass_kernel_spmd(
        nc, in_maps, core_ids=list(range(NCORES)))
    out = np.concatenate([r["out"] for r in res.results], axis=0)
    return out.reshape(B, COUT, H, W)
